# revision 8
# baseline (speedup 1.0000x reference)
"""Trainium2 Bass kernel for a dense pre-norm transformer block.

Reference semantics (B=4, T=2048, D=512, H=8, DH=64, fp32):
    h  = LN(x; g, b)
    q,k,v = per-head projections of h
    att = causal softmax(q k^T / sqrt(D))
    x1 = x + (att v) @ Wproj + bproj          (heads concatenated)
    h2 = LN(x1; g, b)                         (same LN params, faithful to source)
    out = x1 + relu(h2 @ W1 + b1) @ W2 + b2

Sharding: 8 cores = 4 batches x 2 parities. Core (b, p) owns the 8
row-blocks {p, p+2, ..., p+14} (128 rows each) of batch b. Causal key
extents are rounded up to 512 so even/odd block sets see identical
work -> one uniform SPMD program, no collectives. Exact causality is
restored with multiplicative 0/1 masks on the exp() values (host
provides per-parity masks).

The dataflow needs activations feature-major (features on partitions)
for every matmul, but avoids all on-chip transposes (the DMA-transpose
path only allows 2 sync waits per instruction, which Tile's scheduler
exceeds):
  - the host passes x pre-transposed (xbT, x_ownT, bf16);
  - LN statistics are computed row-major (tokens on partitions, cheap
    free-dim reductions), written to a DRAM scratch row, and read back
    with a 0-stride partition-broadcast DMA so they can be applied in
    the transposed domain;
  - h2T is built from a transposed second projection Wproj^T @ o_catT
    plus the transposed residual, instead of transposing x1.
Scores are computed key-major [s, t]; softmax denominators come for
free from an all-ones column appended to v. All matmuls are bf16 with
fp32 PSUM accumulation; residuals, LN stats and softmax normalization
stay fp32.
"""

import os
import sys

sys.path.insert(0, "/opt/trn_rl_repo")

import numpy as np
import ml_dtypes
from contextlib import ExitStack

import concourse.bass as bass
import concourse.bacc as bacc
import concourse.mybir as mybir
import concourse.tile as tile
from concourse.bass_utils import run_bass_kernel_spmd

B, T, D, H = 4, 2048, 512, 8
DH = D // H            # 64
HID = 4 * D            # 2048
P = 128                # partitions
NT = T // P            # 16 row blocks over full T
NQ = 8                 # own row blocks per core
TQ = NQ * P            # 1024 own rows per core
EPS = 1e-5
SCALE = D ** -0.5
F32 = mybir.dt.float32
BF16 = mybir.dt.bfloat16

# first own-block (local index) attending key-block k; extents rounded to 512
JMIN = [0, 0, 0, 0, 2, 2, 2, 2, 4, 4, 4, 4, 6, 6, 6, 6]

_CACHED = {}


def _build_nc():
    nc = bacc.Bacc()

    xb = nc.dram_tensor("xb", [T, D], F32, kind="ExternalInput")
    xbT = nc.dram_tensor("xbT", [D, T], BF16, kind="ExternalInput")
    x_own = nc.dram_tensor("x_own", [TQ, D], F32, kind="ExternalInput")
    x_ownT = nc.dram_tensor("x_ownT", [D, TQ], BF16, kind="ExternalInput")
    wq = nc.dram_tensor("wq", [D, D], BF16, kind="ExternalInput")
    wk = nc.dram_tensor("wk", [D, D], BF16, kind="ExternalInput")
    wv = nc.dram_tensor("wv", [D, D], BF16, kind="ExternalInput")
    wp = nc.dram_tensor("wp", [D, D], BF16, kind="ExternalInput")
    w1 = nc.dram_tensor("w1", [D, HID], BF16, kind="ExternalInput")
    w2 = nc.dram_tensor("w2", [HID, D], BF16, kind="ExternalInput")
    gvec = nc.dram_tensor("gvec", [D], F32, kind="ExternalInput")
    bvec = nc.dram_tensor("bvec", [D], F32, kind="ExternalInput")
    bpro = nc.dram_tensor("bpro", [D], F32, kind="ExternalInput")
    b1v = nc.dram_tensor("b1v", [HID], F32, kind="ExternalInput")
    b2v = nc.dram_tensor("b2v", [D], F32, kind="ExternalInput")
    masks = nc.dram_tensor("masks", [NT, P, 2 * P], BF16, kind="ExternalInput")
    out = nc.dram_tensor("out", [TQ, D], F32, kind="ExternalOutput")

    # DRAM scratch: softmax denominators + LN stat rows (for the
    # partition-broadcast round-trips)
    denbuf = nc.dram_tensor("denbuf", [H, TQ], F32)
    muv = nc.dram_tensor("muv", [T], BF16)
    rsv = nc.dram_tensor("rsv", [T], BF16)
    muov = nc.dram_tensor("muov", [TQ], BF16)
    rsov = nc.dram_tensor("rsov", [TQ], BF16)
    mu2v = nc.dram_tensor("mu2v", [TQ], BF16)
    rs2v = nc.dram_tensor("rs2v", [TQ], BF16)

    with ExitStack() as ctx:
        tc = ctx.enter_context(tile.TileContext(nc))
        consts = ctx.enter_context(tc.tile_pool(name="consts", bufs=1))

        # ---- constants ----------------------------------------------------
        g_sb = consts.tile([P, 4], F32)
        nc.sync.dma_start(out=g_sb, in_=gvec[:].rearrange("(c p) -> p c", p=P))
        b_sb = consts.tile([P, 4], F32)
        nc.sync.dma_start(out=b_sb, in_=bvec[:].rearrange("(c p) -> p c", p=P))
        bpro_sb = consts.tile([P, 4], F32)
        nc.sync.dma_start(
            out=bpro_sb, in_=bpro[:].rearrange("(c p) -> p c", p=P)
        )
        b1_sb = consts.tile([P, 16], F32)
        nc.sync.dma_start(out=b1_sb, in_=b1v[:].rearrange("(c p) -> p c", p=P))
        bpro_bc = consts.tile([P, D], F32)
        nc.gpsimd.dma_start(out=bpro_bc, in_=bpro[:].partition_broadcast(P))
        b2_bc = consts.tile([P, D], F32)
        nc.gpsimd.dma_start(out=b2_bc, in_=b2v[:].partition_broadcast(P))
        eps_sb = consts.tile([P, 1], F32)
        nc.vector.memset(eps_sb, EPS)
        masks_sb = consts.tile([P, NT, 2 * P], BF16)
        nc.sync.dma_start(out=masks_sb, in_=masks[:].transpose([1, 0, 2]))

        # ---- weights (bf16, feature chunks on partitions) -----------------
        wq_sb = consts.tile([P, 4, D], BF16)
        nc.sync.dma_start(out=wq_sb, in_=wq[:].rearrange("(c p) n -> p c n", p=P))
        wk_sb = consts.tile([P, 4, D], BF16)
        nc.sync.dma_start(out=wk_sb, in_=wk[:].rearrange("(c p) n -> p c n", p=P))
        wv_sb = consts.tile([P, 4, D], BF16)
        nc.sync.dma_start(out=wv_sb, in_=wv[:].rearrange("(c p) n -> p c n", p=P))
        wp_sb = consts.tile([P, 4, D], BF16)
        nc.sync.dma_start(out=wp_sb, in_=wp[:].rearrange("(c p) n -> p c n", p=P))
        w1_sb = consts.tile([P, 4, HID], BF16)
        nc.sync.dma_start(out=w1_sb, in_=w1[:].rearrange("(c p) n -> p c n", p=P))
        w2_sb = consts.tile([P, 16, D], BF16)
        nc.sync.dma_start(out=w2_sb, in_=w2[:].rearrange("(c p) n -> p c n", p=P))

        # ---- persistent activations --------------------------------------
        acts = ctx.enter_context(tc.tile_pool(name="acts", bufs=1))
        x1row = acts.tile([P, NQ, D], F32)
        o_catT = acts.tile([P, 4, TQ], BF16)     # (att@v)^T per head-pair
        h2T = acts.tile([P, 4, TQ], BF16)
        x_ownT_sb = acts.tile([P, 4, TQ], BF16)
        nc.sync.dma_start(
            out=x_ownT_sb, in_=x_ownT[:].rearrange("(c p) t -> p c t", p=P)
        )

        # alive through attention (phases 1-3), freed before FFN
        qkv_pool = ctx.enter_context(tc.tile_pool(name="qkv_pool", bufs=1))
        qT = qkv_pool.tile([P, 4, TQ], BF16)     # own columns only, compact
        kT = qkv_pool.tile([P, 4, T], BF16)
        v_aug = qkv_pool.tile([P, NT, H, DH + 1], BF16)   # v + ones column

        def ln_stats(x_tile, mus, rss, it, stat_pool):
            """Row-major LN stats of x_tile [128, D] -> mus/rss col it."""
            stats = stat_pool.tile([P, nc.vector.BN_STATS_DIM], F32, tag="st")
            nc.vector.bn_stats(out=stats, in_=x_tile)
            mv = stat_pool.tile([P, nc.vector.BN_AGGR_DIM], F32, tag="mv")
            nc.vector.bn_aggr(out=mv, in_=stats)
            nc.vector.tensor_copy(mus[:, it:it + 1], mv[:, 0:1])
            rstd = stat_pool.tile([P, 1], F32, tag="rs")
            nc.scalar.activation(
                out=rstd, in_=mv[:, 1:2],
                func=mybir.ActivationFunctionType.Sqrt,
                bias=eps_sb, scale=1.0,
            )
            nc.vector.reciprocal(out=rstd, in_=rstd)
            nc.vector.tensor_copy(rss[:, it:it + 1], rstd)

        def ln_apply_T(dst, src_c, mu_b, rs_b, c):
            """dst[:,c,:] = ((src - mu)*rstd)*g + b, transposed domain."""
            nc.vector.tensor_sub(dst[:, c, :], src_c, mu_b)
            nc.vector.tensor_mul(dst[:, c, :], dst[:, c, :], rs_b)
            nc.vector.tensor_scalar(
                out=dst[:, c, :], in0=dst[:, c, :],
                scalar1=g_sb[:, c:c + 1], scalar2=b_sb[:, c:c + 1],
                op0=mybir.AluOpType.mult, op1=mybir.AluOpType.add,
            )

        # ---- phases 1+2: LN1 -> hT -> q/k/v ------------------------------
        with ExitStack() as p12:
            hT_pool = p12.enter_context(tc.tile_pool(name="hT_pool", bufs=1))
            hT = hT_pool.tile([P, 4, T], BF16)       # LN(x)^T, full batch
            hT_own = hT_pool.tile([P, 4, TQ], BF16)  # LN(x)^T, own rows
            xbT_sb = hT_pool.tile([P, 4, T], BF16)
            nc.sync.dma_start(
                out=xbT_sb, in_=xbT[:].rearrange("(c p) t -> p c t", p=P)
            )
            stat1 = p12.enter_context(tc.tile_pool(name="stat1", bufs=4))
            xpool = p12.enter_context(tc.tile_pool(name="xpool", bufs=3))
            spool = p12.enter_context(tc.tile_pool(name="spool", bufs=1))
            bpool = p12.enter_context(tc.tile_pool(name="bpool", bufs=1))

            mus = spool.tile([P, NT], BF16)
            rss = spool.tile([P, NT], BF16)
            for it in range(NT):
                x_tile = xpool.tile([P, D], F32, tag="x")
                nc.sync.dma_start(out=x_tile, in_=xb[it * P:(it + 1) * P, :])
                ln_stats(x_tile, mus, rss, it, stat1)
            nc.sync.dma_start(out=muv[:].rearrange("(c p) -> p c", p=P), in_=mus)
            nc.sync.dma_start(out=rsv[:].rearrange("(c p) -> p c", p=P), in_=rss)

            muso = spool.tile([P, NQ], BF16)
            rsso = spool.tile([P, NQ], BF16)
            for it in range(NQ):
                x_tile = xpool.tile([P, D], F32, tag="x")
                nc.sync.dma_start(
                    out=x_tile, in_=x_own[it * P:(it + 1) * P, :]
                )
                ln_stats(x_tile, muso, rsso, it, stat1)
            nc.sync.dma_start(
                out=muov[:].rearrange("(c p) -> p c", p=P), in_=muso
            )
            nc.sync.dma_start(
                out=rsov[:].rearrange("(c p) -> p c", p=P), in_=rsso
            )

            mu_b = bpool.tile([P, T], BF16)
            nc.gpsimd.dma_start(out=mu_b, in_=muv[:].partition_broadcast(P))
            rs_b = bpool.tile([P, T], BF16)
            nc.gpsimd.dma_start(out=rs_b, in_=rsv[:].partition_broadcast(P))
            muo_b = bpool.tile([P, TQ], BF16)
            nc.gpsimd.dma_start(out=muo_b, in_=muov[:].partition_broadcast(P))
            rso_b = bpool.tile([P, TQ], BF16)
            nc.gpsimd.dma_start(out=rso_b, in_=rsov[:].partition_broadcast(P))

            for c in range(4):
                ln_apply_T(hT, xbT_sb[:, c, :], mu_b, rs_b, c)
                ln_apply_T(hT_own, x_ownT_sb[:, c, :], muo_b, rso_b, c)

            # ---- qT / kT / v ---------------------------------------------
            qkv_ps = p12.enter_context(
                tc.tile_pool(name="qkv_ps", bufs=4, space="PSUM")
            )
            for pair in range(4):
                for ts_ in range(2):
                    sl = slice(ts_ * 512, (ts_ + 1) * 512)
                    ps_q = qkv_ps.tile([P, 512], F32, tag="ps")
                    for c in range(4):
                        nc.tensor.matmul(
                            ps_q,
                            wq_sb[:, c, pair * P:(pair + 1) * P],
                            hT_own[:, c, sl],
                            start=(c == 0), stop=(c == 3),
                        )
                    nc.any.tensor_copy(qT[:, pair, sl], ps_q)
                for ts_ in range(4):
                    sl = slice(ts_ * 512, (ts_ + 1) * 512)
                    ps_k = qkv_ps.tile([P, 512], F32, tag="ps")
                    for c in range(4):
                        nc.tensor.matmul(
                            ps_k,
                            wk_sb[:, c, pair * P:(pair + 1) * P],
                            hT[:, c, sl],
                            start=(c == 0), stop=(c == 3),
                        )
                    nc.any.tensor_copy(kT[:, pair, sl], ps_k)
            for st in range(NT):
                ps_v = qkv_ps.tile([P, 512], F32, tag="ps")
                for c in range(4):
                    nc.tensor.matmul(
                        ps_v,
                        hT[:, c, st * P:(st + 1) * P],
                        wv_sb[:, c, :],
                        start=(c == 0), stop=(c == 3),
                    )
                nc.any.tensor_copy(
                    v_aug[:, st, :, 0:DH],
                    ps_v.rearrange("p (h e) -> p h e", h=H),
                )
                nc.vector.memset(v_aug[:, st, :, DH:DH + 1], 1.0)

        # ---- phase 3: attention ------------------------------------------
        with ExitStack() as p3:
            sc_ps = p3.enter_context(
                tc.tile_pool(name="sc_ps", bufs=2, space="PSUM")
            )
            av_ps = p3.enter_context(
                tc.tile_pool(name="av_ps", bufs=4, space="PSUM")
            )
            epool = p3.enter_context(tc.tile_pool(name="epool", bufs=3))
            dpool = p3.enter_context(tc.tile_pool(name="dpool", bufs=3))
            for h in range(H):
                pair, half = h // 2, h % 2
                pr = slice(half * DH, (half + 1) * DH)
                oc0 = av_ps.tile([P, 512], F32, tag="oc")
                oc1 = av_ps.tile([P, 512], F32, tag="oc")
                for k in range(NT):
                    ss = P * JMIN[k]
                    L = TQ - ss
                    sco = sc_ps.tile([P, 1024], F32, tag="sc")
                    for n0 in range(0, L, 512):
                        nn = min(512, L - n0)
                        nc.tensor.matmul(
                            sco[:, n0:n0 + nn],
                            kT[pr, pair, k * P:(k + 1) * P],
                            qT[pr, pair, ss + n0:ss + n0 + nn],
                            start=True, stop=True,
                        )
                    ex = epool.tile([P, 1024], BF16, tag="ex")
                    nc.scalar.activation(
                        out=ex[:, 0:L], in_=sco[:, 0:L],
                        func=mybir.ActivationFunctionType.Exp,
                        scale=SCALE,
                    )
                    nc.vector.tensor_mul(
                        ex[:, 0:2 * P], ex[:, 0:2 * P], masks_sb[:, k, :]
                    )
                    lhs_v = v_aug[:, k, h, :]
                    if ss < 512:
                        nc.tensor.matmul(
                            oc0[0:DH + 1, ss:512],
                            lhs_v,
                            ex[:, 0:512 - ss],
                            start=(k == 0), stop=(k == 7),
                        )
                        nc.tensor.matmul(
                            oc1[0:DH + 1, :],
                            lhs_v,
                            ex[:, 512 - ss:L],
                            start=(k == 0), stop=(k == 15),
                        )
                    else:
                        nc.tensor.matmul(
                            oc1[0:DH + 1, ss - 512:512],
                            lhs_v,
                            ex[:, 0:L],
                            start=False, stop=(k == 15),
                        )
                # softmax denominators -> 1/den, broadcast over 64 partitions
                # (via a DRAM round-trip; SBUF sources reject 0-stride APs)
                den = dpool.tile([1, TQ], F32, tag="den")
                nc.vector.tensor_copy(den[:, 0:512], oc0[DH:DH + 1, :])
                nc.vector.tensor_copy(den[:, 512:TQ], oc1[DH:DH + 1, :])
                nc.vector.reciprocal(den, den)
                nc.gpsimd.dma_start(out=denbuf[h, :], in_=den)
                invb = dpool.tile([DH, TQ], F32, tag="invb")
                nc.gpsimd.dma_start(
                    out=invb, in_=denbuf[h, :].partition_broadcast(DH)
                )
                nc.vector.tensor_mul(
                    o_catT[pr, pair, 0:512], oc0[0:DH, :], invb[:, 0:512]
                )
                nc.vector.tensor_mul(
                    o_catT[pr, pair, 512:TQ], oc1[0:DH, :], invb[:, 512:TQ]
                )

        # ---- phases 4+5: proj (both orientations), residual, LN2 ---------
        with ExitStack() as p45:
            x1T_pool = p45.enter_context(tc.tile_pool(name="x1T_pool", bufs=1))
            x1T = x1T_pool.tile([P, 4, TQ], BF16)
            pr_ps = p45.enter_context(
                tc.tile_pool(name="pr_ps", bufs=2, space="PSUM")
            )
            prT_ps = p45.enter_context(
                tc.tile_pool(name="prT_ps", bufs=2, space="PSUM")
            )
            xopool = p45.enter_context(tc.tile_pool(name="xopool", bufs=3))
            stat2 = p45.enter_context(tc.tile_pool(name="stat2", bufs=4))
            spool2 = p45.enter_context(tc.tile_pool(name="spool2", bufs=1))
            bpool2 = p45.enter_context(tc.tile_pool(name="bpool2", bufs=1))

            mu2s = spool2.tile([P, NQ], BF16)
            rs2s = spool2.tile([P, NQ], BF16)
            for tb in range(NQ):
                xo = xopool.tile([P, D], F32, tag="xo")
                nc.sync.dma_start(out=xo, in_=x_own[tb * P:(tb + 1) * P, :])
                ps = pr_ps.tile([P, D], F32, tag="pp")
                for pair in range(4):
                    nc.tensor.matmul(
                        ps,
                        o_catT[:, pair, tb * P:(tb + 1) * P],
                        wp_sb[:, pair, :],
                        start=(pair == 0), stop=(pair == 3),
                    )
                nc.vector.tensor_add(x1row[:, tb, :], ps, xo)
                nc.vector.tensor_add(x1row[:, tb, :], x1row[:, tb, :], bpro_bc)
                ln_stats(x1row[:, tb, :], mu2s, rs2s, tb, stat2)
            nc.sync.dma_start(
                out=mu2v[:].rearrange("(c p) -> p c", p=P), in_=mu2s
            )
            nc.sync.dma_start(
                out=rs2v[:].rearrange("(c p) -> p c", p=P), in_=rs2s
            )

            # transposed projection: x1T = x_ownT + Wproj^T @ o_catT + bproj
            for dt in range(4):
                for tch in range(2):
                    sl = slice(tch * 512, (tch + 1) * 512)
                    psT = prT_ps.tile([P, 512], F32, tag="pt")
                    for pair in range(4):
                        nc.tensor.matmul(
                            psT,
                            wp_sb[:, pair, dt * P:(dt + 1) * P],
                            o_catT[:, pair, sl],
                            start=(pair == 0), stop=(pair == 3),
                        )
                    nc.vector.tensor_scalar(
                        out=x1T[:, dt, sl], in0=psT,
                        scalar1=bpro_sb[:, dt:dt + 1], scalar2=None,
                        op0=mybir.AluOpType.add,
                    )
                    nc.vector.tensor_add(
                        x1T[:, dt, sl], x1T[:, dt, sl], x_ownT_sb[:, dt, sl]
                    )

            mu2_b = bpool2.tile([P, TQ], BF16)
            nc.gpsimd.dma_start(out=mu2_b, in_=mu2v[:].partition_broadcast(P))
            rs2_b = bpool2.tile([P, TQ], BF16)
            nc.gpsimd.dma_start(out=rs2_b, in_=rs2v[:].partition_broadcast(P))
            for c in range(4):
                ln_apply_T(h2T, x1T[:, c, :], mu2_b, rs2_b, c)

        # ---- phase 6: FFN + residual + store -----------------------------
        with ExitStack() as p6:
            f1_ps = p6.enter_context(
                tc.tile_pool(name="f1_ps", bufs=3, space="PSUM")
            )
            f2_ps = p6.enter_context(
                tc.tile_pool(name="f2_ps", bufs=2, space="PSUM")
            )
            fpool = p6.enter_context(tc.tile_pool(name="fpool", bufs=18))
            opool = p6.enter_context(tc.tile_pool(name="opool", bufs=3))
            for tch in range(2):
                tsl = slice(tch * 512, (tch + 1) * 512)
                ff1 = []
                for ht in range(16):
                    ps = f1_ps.tile([P, 512], F32, tag="f1")
                    for c in range(4):
                        nc.tensor.matmul(
                            ps,
                            w1_sb[:, c, ht * P:(ht + 1) * P],
                            h2T[:, c, tsl],
                            start=(c == 0), stop=(c == 3),
                        )
                    f1s = fpool.tile([P, 512], BF16, tag="f1s")
                    nc.vector.tensor_scalar(
                        out=f1s, in0=ps,
                        scalar1=b1_sb[:, ht:ht + 1], scalar2=0.0,
                        op0=mybir.AluOpType.add, op1=mybir.AluOpType.max,
                    )
                    ff1.append(f1s)
                for tbl in range(4):
                    tb = tch * 4 + tbl
                    ps2 = f2_ps.tile([P, D], F32, tag="f2")
                    for ht in range(16):
                        nc.tensor.matmul(
                            ps2,
                            ff1[ht][:, tbl * P:(tbl + 1) * P],
                            w2_sb[:, ht, :],
                            start=(ht == 0), stop=(ht == 15),
                        )
                    orow = opool.tile([P, D], F32, tag="or")
                    nc.vector.tensor_add(orow, ps2, x1row[:, tb, :])
                    nc.vector.tensor_add(orow, orow, b2_bc)
                    nc.sync.dma_start(
                        out=out[tb * P:(tb + 1) * P, :], in_=orow
                    )
    nc.compile()
    return nc


def _make_masks(parity: int) -> np.ndarray:
    """[NT, 128, 256] multiplicative masks restoring exact causality."""
    m = np.zeros((NT, P, 2 * P), np.float32)
    for k in range(NT):
        jmin = JMIN[k]
        for a in range(2):
            g = 2 * (jmin + a) + parity
            t_glob = g * P + np.arange(P)[None, :]
            s_glob = k * P + np.arange(P)[:, None]
            m[k, :, a * P:(a + 1) * P] = (t_glob >= s_glob).astype(np.float32)
    return m.astype(ml_dtypes.bfloat16)


def _prep(inputs):
    f32 = lambda a: np.ascontiguousarray(np.asarray(a, dtype=np.float32))
    bf = lambda a: np.ascontiguousarray(
        np.asarray(a, dtype=np.float32).astype(ml_dtypes.bfloat16)
    )
    x = f32(inputs["x"])
    # [H, D, DH] -> [D, H*DH] with column h*DH+e
    wq = bf(np.asarray(inputs["Wq"], np.float32).transpose(1, 0, 2).reshape(D, D))
    wk = bf(np.asarray(inputs["Wk"], np.float32).transpose(1, 0, 2).reshape(D, D))
    wv = bf(np.asarray(inputs["Wv"], np.float32).transpose(1, 0, 2).reshape(D, D))
    common = {
        "wq": wq, "wk": wk, "wv": wv,
        "wp": bf(inputs["Wproj"]),
        "w1": bf(inputs["W1"]),
        "w2": bf(inputs["W2"]),
        "gvec": f32(inputs["ln1_g"]),
        "bvec": f32(inputs["ln1_b"]),
        "bpro": f32(inputs["bproj"]),
        "b1v": f32(inputs["b1"]),
        "b2v": f32(inputs["b2"]),
    }
    masks = [_make_masks(0), _make_masks(1)]
    in_maps = []
    for c in range(8):
        b, p = c // 2, c % 2
        xb = np.ascontiguousarray(x[b])
        xo = np.ascontiguousarray(
            x[b].reshape(NT, P, D)[p::2].reshape(TQ, D)
        )
        in_maps.append(dict(
            common,
            xb=xb,
            xbT=bf(xb.T),
            x_own=xo,
            x_ownT=bf(xo.T),
            masks=masks[p],
        ))
    return in_maps


def _run(inputs, trace=False):
    if "nc" not in _CACHED:
        _CACHED["nc"] = _build_nc()
    nc = _CACHED["nc"]
    in_maps = _prep(inputs)
    res = run_bass_kernel_spmd(nc, in_maps, core_ids=list(range(8)), trace=trace)
    out = np.empty((B, T, D), np.float32)
    for c in range(8):
        b, p = c // 2, c % 2
        out[b].reshape(NT, P, D)[p::2] = res.results[c]["out"].reshape(NQ, P, D)
    return out, res


def kernel(**inputs) -> np.ndarray:
    out, _ = _run(inputs, trace=False)
    return out


# revision 14
# speedup vs baseline: 1.0362x; 1.0362x over previous
"""Trainium2 Bass kernel for a dense pre-norm transformer block.

Reference semantics (B=4, T=2048, D=512, H=8, DH=64, fp32):
    h  = LN(x; g, b)
    q,k,v = per-head projections of h
    att = causal softmax(q k^T / sqrt(D))
    x1 = x + (att v) @ Wproj + bproj          (heads concatenated)
    h2 = LN(x1; g, b)                         (same LN params, faithful to source)
    out = x1 + relu(h2 @ W1 + b1) @ W2 + b2

Sharding: 8 cores = 4 batches x 2 parities. Core (b, p) owns the 8
row-blocks {p, p+2, ..., p+14} (128 rows each) of batch b. Causal key
extents are rounded up to 512 so even/odd block sets see identical
work -> one uniform SPMD program, no collectives. Exact causality is
restored with multiplicative 0/1 masks on the exp() values (host
provides per-parity masks).

The dataflow needs activations feature-major (features on partitions)
for every matmul, but avoids all on-chip transposes (the DMA-transpose
path only allows 2 sync waits per instruction, which Tile's scheduler
exceeds):
  - the host passes x pre-transposed (xbT, x_ownT, bf16);
  - LN statistics are computed row-major (tokens on partitions, cheap
    free-dim reductions), written to a DRAM scratch row, and read back
    with a 0-stride partition-broadcast DMA so they can be applied in
    the transposed domain;
  - h2T is built from a transposed second projection Wproj^T @ o_catT
    plus the transposed residual, instead of transposing x1.
Scores are computed key-major [s, t]; softmax denominators come for
free from an all-ones column appended to v. All matmuls are bf16 with
fp32 PSUM accumulation; residuals, LN stats and softmax normalization
stay fp32.
"""

import os
import sys

sys.path.insert(0, "/opt/trn_rl_repo")

import numpy as np
import ml_dtypes
from contextlib import ExitStack

import concourse.bass as bass
import concourse.bacc as bacc
import concourse.mybir as mybir
import concourse.tile as tile
from concourse.bass_utils import run_bass_kernel_spmd

B, T, D, H = 4, 2048, 512, 8
DH = D // H            # 64
HID = 4 * D            # 2048
P = 128                # partitions
NT = T // P            # 16 row blocks over full T
NQ = 8                 # own row blocks per core
TQ = NQ * P            # 1024 own rows per core
EPS = 1e-5
SCALE = D ** -0.5
F32 = mybir.dt.float32
BF16 = mybir.dt.bfloat16

# first own-block (local index) attending key-block k; extents rounded to 512
JMIN = [0, 0, 0, 0, 2, 2, 2, 2, 4, 4, 4, 4, 6, 6, 6, 6]

_CACHED = {}


def _build_nc():
    nc = bacc.Bacc()

    xb = nc.dram_tensor("xb", [T, D], F32, kind="ExternalInput")
    xbT = nc.dram_tensor("xbT", [D, T], BF16, kind="ExternalInput")
    x_own = nc.dram_tensor("x_own", [TQ, D], F32, kind="ExternalInput")
    x_ownT = nc.dram_tensor("x_ownT", [D, TQ], BF16, kind="ExternalInput")
    wq = nc.dram_tensor("wq", [D, D], BF16, kind="ExternalInput")
    wk = nc.dram_tensor("wk", [D, D], BF16, kind="ExternalInput")
    wv = nc.dram_tensor("wv", [D, D], BF16, kind="ExternalInput")
    wp = nc.dram_tensor("wp", [D, D], BF16, kind="ExternalInput")
    w1 = nc.dram_tensor("w1", [D, HID], BF16, kind="ExternalInput")
    w2 = nc.dram_tensor("w2", [HID, D], BF16, kind="ExternalInput")
    gvec = nc.dram_tensor("gvec", [D], F32, kind="ExternalInput")
    bvec = nc.dram_tensor("bvec", [D], F32, kind="ExternalInput")
    bpro = nc.dram_tensor("bpro", [D], F32, kind="ExternalInput")
    b1v = nc.dram_tensor("b1v", [HID], F32, kind="ExternalInput")
    b2v = nc.dram_tensor("b2v", [D], F32, kind="ExternalInput")
    masks = nc.dram_tensor("masks", [NT, P, 2 * P], BF16, kind="ExternalInput")
    out = nc.dram_tensor("out", [TQ, D], F32, kind="ExternalOutput")

    # DRAM scratch: softmax denominators + LN stat rows (for the
    # partition-broadcast round-trips)
    denbuf = nc.dram_tensor("denbuf", [H, TQ], F32)
    muv = nc.dram_tensor("muv", [T], BF16)
    rsv = nc.dram_tensor("rsv", [T], BF16)
    muov = nc.dram_tensor("muov", [TQ], BF16)
    rsov = nc.dram_tensor("rsov", [TQ], BF16)
    mu2v = nc.dram_tensor("mu2v", [TQ], BF16)
    rs2v = nc.dram_tensor("rs2v", [TQ], BF16)

    with ExitStack() as ctx:
        tc = ctx.enter_context(tile.TileContext(nc))
        consts = ctx.enter_context(tc.tile_pool(name="consts", bufs=1))

        # ---- constants ----------------------------------------------------
        g_sb = consts.tile([P, 4], F32)
        nc.sync.dma_start(out=g_sb, in_=gvec[:].rearrange("(c p) -> p c", p=P))
        b_sb = consts.tile([P, 4], F32)
        nc.sync.dma_start(out=b_sb, in_=bvec[:].rearrange("(c p) -> p c", p=P))
        bpro_sb = consts.tile([P, 4], F32)
        nc.sync.dma_start(
            out=bpro_sb, in_=bpro[:].rearrange("(c p) -> p c", p=P)
        )
        b1_sb = consts.tile([P, 16], F32)
        nc.sync.dma_start(out=b1_sb, in_=b1v[:].rearrange("(c p) -> p c", p=P))
        bpro_bc = consts.tile([P, D], F32)
        nc.gpsimd.dma_start(out=bpro_bc, in_=bpro[:].partition_broadcast(P))
        b2_bc = consts.tile([P, D], F32)
        nc.gpsimd.dma_start(out=b2_bc, in_=b2v[:].partition_broadcast(P))
        eps_sb = consts.tile([P, 1], F32)
        nc.vector.memset(eps_sb, EPS)
        masks_sb = consts.tile([P, NT, 2 * P], BF16)
        nc.sync.dma_start(out=masks_sb, in_=masks[:].transpose([1, 0, 2]))

        # ---- weights (bf16, feature chunks on partitions) -----------------
        wq_sb = consts.tile([P, 4, D], BF16)
        nc.sync.dma_start(out=wq_sb, in_=wq[:].rearrange("(c p) n -> p c n", p=P))
        wk_sb = consts.tile([P, 4, D], BF16)
        nc.sync.dma_start(out=wk_sb, in_=wk[:].rearrange("(c p) n -> p c n", p=P))
        wv_sb = consts.tile([P, 4, D], BF16)
        nc.sync.dma_start(out=wv_sb, in_=wv[:].rearrange("(c p) n -> p c n", p=P))
        wp_sb = consts.tile([P, 4, D], BF16)
        nc.sync.dma_start(out=wp_sb, in_=wp[:].rearrange("(c p) n -> p c n", p=P))
        w1_sb = consts.tile([P, 4, HID], BF16)
        nc.sync.dma_start(out=w1_sb, in_=w1[:].rearrange("(c p) n -> p c n", p=P))
        w2_sb = consts.tile([P, 16, D], BF16)
        nc.sync.dma_start(out=w2_sb, in_=w2[:].rearrange("(c p) n -> p c n", p=P))

        # ---- persistent activations --------------------------------------
        acts = ctx.enter_context(tc.tile_pool(name="acts", bufs=1))
        x1row = acts.tile([P, NQ, D], F32)
        o_catT = acts.tile([P, 4, TQ], BF16)     # (att@v)^T per head-pair
        h2T = acts.tile([P, 4, TQ], BF16)
        x_ownT_sb = acts.tile([P, 4, TQ], BF16)
        nc.sync.dma_start(
            out=x_ownT_sb, in_=x_ownT[:].rearrange("(c p) t -> p c t", p=P)
        )

        # alive through attention (phases 1-3), freed before FFN
        qkv_pool = ctx.enter_context(tc.tile_pool(name="qkv_pool", bufs=1))
        qT = qkv_pool.tile([P, 4, TQ], BF16)     # own columns only, compact
        kT = qkv_pool.tile([P, 4, T], BF16)
        v_aug = qkv_pool.tile([P, NT, H, DH + 1], BF16)   # v + ones column

        def ln_stats(x_tile, mus, vs_, it, stat_pool):
            """Row-major LN stats of x_tile [128, D] -> mu (bf16) + var col."""
            stats = stat_pool.tile([P, nc.vector.BN_STATS_DIM], F32, tag="st")
            nc.vector.bn_stats(out=stats, in_=x_tile)
            mv = stat_pool.tile([P, nc.vector.BN_AGGR_DIM], F32, tag="mv")
            nc.vector.bn_aggr(out=mv, in_=stats)
            nc.vector.tensor_copy(mus[:, it:it + 1], mv[:, 0:1])
            nc.vector.tensor_copy(vs_[:, it:it + 1], mv[:, 1:2])

        def ln_finish(vs_, rss):
            """rss (bf16) = 1/sqrt(vs_ + eps), one batched op chain."""
            nc.scalar.activation(
                out=vs_, in_=vs_,
                func=mybir.ActivationFunctionType.Sqrt,
                bias=eps_sb, scale=1.0,
            )
            nc.vector.reciprocal(out=vs_, in_=vs_)
            nc.vector.tensor_copy(rss, vs_)

        def ln_apply_T(dst, src_c, mu_b, rs_b, c):
            """dst[:,c,:] = ((src - mu)*rstd)*g + b, transposed domain."""
            nc.vector.tensor_sub(dst[:, c, :], src_c, mu_b)
            nc.vector.tensor_mul(dst[:, c, :], dst[:, c, :], rs_b)
            nc.vector.tensor_scalar(
                out=dst[:, c, :], in0=dst[:, c, :],
                scalar1=g_sb[:, c:c + 1], scalar2=b_sb[:, c:c + 1],
                op0=mybir.AluOpType.mult, op1=mybir.AluOpType.add,
            )

        # ---- phases 1+2: LN1 -> hT -> q/k/v ------------------------------
        with ExitStack() as p12:
            hT_pool = p12.enter_context(tc.tile_pool(name="hT_pool", bufs=1))
            hT = hT_pool.tile([P, 4, T], BF16)       # LN(x)^T, full batch
            hT_own = hT_pool.tile([P, 4, TQ], BF16)  # LN(x)^T, own rows
            xbT_sb = hT_pool.tile([P, 4, T], BF16)
            nc.sync.dma_start(
                out=xbT_sb, in_=xbT[:].rearrange("(c p) t -> p c t", p=P)
            )
            stat1 = p12.enter_context(tc.tile_pool(name="stat1", bufs=8))
            xpool = p12.enter_context(tc.tile_pool(name="xpool", bufs=6))
            spool = p12.enter_context(tc.tile_pool(name="spool", bufs=1))
            bpool = p12.enter_context(tc.tile_pool(name="bpool", bufs=1))

            muso = spool.tile([P, NQ], BF16)
            vso = spool.tile([P, NQ], F32)
            rsso = spool.tile([P, NQ], BF16)
            for it in range(NQ):
                x_tile = xpool.tile([P, D], F32, tag="x")
                nc.sync.dma_start(
                    out=x_tile, in_=x_own[it * P:(it + 1) * P, :]
                )
                ln_stats(x_tile, muso, vso, it, stat1)
            ln_finish(vso, rsso)
            nc.sync.dma_start(
                out=muov[:].rearrange("(c p) -> p c", p=P), in_=muso
            )
            nc.sync.dma_start(
                out=rsov[:].rearrange("(c p) -> p c", p=P), in_=rsso
            )
            muo_b = bpool.tile([P, TQ], BF16)
            nc.gpsimd.dma_start(out=muo_b, in_=muov[:].partition_broadcast(P))
            rso_b = bpool.tile([P, TQ], BF16)
            nc.gpsimd.dma_start(out=rso_b, in_=rsov[:].partition_broadcast(P))
            for c in range(4):
                ln_apply_T(hT_own, x_ownT_sb[:, c, :], muo_b, rso_b, c)

            mus = spool.tile([P, NT], BF16)
            vs1 = spool.tile([P, NT], F32)
            rss = spool.tile([P, NT], BF16)
            for it in range(NT):
                x_tile = xpool.tile([P, D], F32, tag="x")
                nc.sync.dma_start(out=x_tile, in_=xb[it * P:(it + 1) * P, :])
                ln_stats(x_tile, mus, vs1, it, stat1)
            ln_finish(vs1, rss)
            nc.sync.dma_start(out=muv[:].rearrange("(c p) -> p c", p=P), in_=mus)
            nc.sync.dma_start(out=rsv[:].rearrange("(c p) -> p c", p=P), in_=rss)
            mu_b = bpool.tile([P, T], BF16)
            nc.gpsimd.dma_start(out=mu_b, in_=muv[:].partition_broadcast(P))
            rs_b = bpool.tile([P, T], BF16)
            nc.gpsimd.dma_start(out=rs_b, in_=rsv[:].partition_broadcast(P))
            for c in range(4):
                ln_apply_T(hT, xbT_sb[:, c, :], mu_b, rs_b, c)

            # ---- qT / kT / v ---------------------------------------------
            qkv_ps = p12.enter_context(
                tc.tile_pool(name="qkv_ps", bufs=4, space="PSUM")
            )
            for pair in range(4):
                for ts_ in range(2):
                    sl = slice(ts_ * 512, (ts_ + 1) * 512)
                    ps_q = qkv_ps.tile([P, 512], F32, tag="ps")
                    for c in range(4):
                        nc.tensor.matmul(
                            ps_q,
                            wq_sb[:, c, pair * P:(pair + 1) * P],
                            hT_own[:, c, sl],
                            start=(c == 0), stop=(c == 3),
                        )
                    nc.any.tensor_copy(qT[:, pair, sl], ps_q)
            for ts_ in range(4):
                sl = slice(ts_ * 512, (ts_ + 1) * 512)
                for pair in range(4):
                    ps_k = qkv_ps.tile([P, 512], F32, tag="ps")
                    for c in range(4):
                        nc.tensor.matmul(
                            ps_k,
                            wk_sb[:, c, pair * P:(pair + 1) * P],
                            hT[:, c, sl],
                            start=(c == 0), stop=(c == 3),
                        )
                    nc.any.tensor_copy(kT[:, pair, sl], ps_k)
                for st in range(4 * ts_, 4 * ts_ + 4):
                    ps_v = qkv_ps.tile([P, 512], F32, tag="ps")
                    for c in range(4):
                        nc.tensor.matmul(
                            ps_v,
                            hT[:, c, st * P:(st + 1) * P],
                            wv_sb[:, c, :],
                            start=(c == 0), stop=(c == 3),
                        )
                    nc.any.tensor_copy(
                        v_aug[:, st, :, 0:DH],
                        ps_v.rearrange("p (h e) -> p h e", h=H),
                    )
                    nc.vector.memset(v_aug[:, st, :, DH:DH + 1], 1.0)

        # ---- phase 3: attention (head pairs; scores run row-tiled
        # concurrently on the PE for the two heads of a pair) --------------
        with ExitStack() as p3:
            sc_ps = p3.enter_context(
                tc.tile_pool(name="sc_ps", bufs=2, space="PSUM")
            )
            av_ps = p3.enter_context(
                tc.tile_pool(name="av_ps", bufs=4, space="PSUM")
            )
            epool = p3.enter_context(tc.tile_pool(name="epool", bufs=8))
            dpool = p3.enter_context(tc.tile_pool(name="dpool", bufs=4))
            for pair in range(4):
                prs = [slice(0, DH), slice(DH, 2 * DH)]
                oc00 = av_ps.tile([P, 512], F32, tag="oc")
                oc01 = av_ps.tile([P, 512], F32, tag="oc")
                oc10 = av_ps.tile([P, 512], F32, tag="oc")
                oc11 = av_ps.tile([P, 512], F32, tag="oc")
                ocs = [[oc00, oc01], [oc10, oc11]]   # [half][chunk]
                for k in range(NT):
                    ss = P * JMIN[k]
                    L = TQ - ss
                    sco0 = sc_ps.tile([P, 1024], F32, tag="sc")
                    sco1 = sc_ps.tile([P, 1024], F32, tag="sc")
                    scos = [sco0, sco1]
                    for half in range(2):
                        for n0 in range(0, L, 512):
                            nn = min(512, L - n0)
                            nc.tensor.matmul(
                                scos[half][:, n0:n0 + nn],
                                kT[prs[half], pair, k * P:(k + 1) * P],
                                qT[prs[half], pair, ss + n0:ss + n0 + nn],
                                start=True, stop=True,
                            )
                    for half in range(2):
                        h = 2 * pair + half
                        oc0, oc1 = ocs[half]
                        ex = epool.tile([P, 1024], BF16, tag="ex")
                        nc.scalar.activation(
                            out=ex[:, 0:L], in_=scos[half][:, 0:L],
                            func=mybir.ActivationFunctionType.Exp,
                            scale=SCALE,
                        )
                        nc.vector.tensor_mul(
                            ex[:, 0:2 * P], ex[:, 0:2 * P], masks_sb[:, k, :]
                        )
                        lhs_v = v_aug[:, k, h, :]
                        if ss < 512:
                            nc.tensor.matmul(
                                oc0[0:DH + 1, ss:512],
                                lhs_v,
                                ex[:, 0:512 - ss],
                                start=(k == 0), stop=(k == 7),
                            )
                            nc.tensor.matmul(
                                oc1[0:DH + 1, :],
                                lhs_v,
                                ex[:, 512 - ss:L],
                                start=(k == 0), stop=(k == 15),
                            )
                        else:
                            nc.tensor.matmul(
                                oc1[0:DH + 1, ss - 512:512],
                                lhs_v,
                                ex[:, 0:L],
                                start=False, stop=(k == 15),
                            )
                # softmax denominators -> 1/den, broadcast over 64 partitions
                # (via a DRAM round-trip; SBUF sources reject 0-stride APs)
                for half in range(2):
                    h = 2 * pair + half
                    pr = prs[half]
                    oc0, oc1 = ocs[half]
                    den = dpool.tile([1, TQ], F32, tag="den")
                    nc.vector.reciprocal(den[:, 0:512], oc0[DH:DH + 1, :])
                    nc.vector.reciprocal(den[:, 512:TQ], oc1[DH:DH + 1, :])
                    nc.gpsimd.dma_start(out=denbuf[h, :], in_=den)
                    invb = dpool.tile([DH, TQ], F32, tag="invb")
                    nc.gpsimd.dma_start(
                        out=invb, in_=denbuf[h, :].partition_broadcast(DH)
                    )
                    nc.vector.tensor_mul(
                        o_catT[pr, pair, 0:512], oc0[0:DH, :], invb[:, 0:512]
                    )
                    nc.vector.tensor_mul(
                        o_catT[pr, pair, 512:TQ], oc1[0:DH, :],
                        invb[:, 512:TQ]
                    )

        # ---- phases 4+5: proj (both orientations), residual, LN2 ---------
        with ExitStack() as p45:
            x1T_pool = p45.enter_context(tc.tile_pool(name="x1T_pool", bufs=1))
            x1T = x1T_pool.tile([P, 4, TQ], BF16)
            pr_ps = p45.enter_context(
                tc.tile_pool(name="pr_ps", bufs=2, space="PSUM")
            )
            prT_ps = p45.enter_context(
                tc.tile_pool(name="prT_ps", bufs=2, space="PSUM")
            )
            xopool = p45.enter_context(tc.tile_pool(name="xopool", bufs=6))
            stat2 = p45.enter_context(tc.tile_pool(name="stat2", bufs=8))
            spool2 = p45.enter_context(tc.tile_pool(name="spool2", bufs=1))
            bpool2 = p45.enter_context(tc.tile_pool(name="bpool2", bufs=1))

            mu2s = spool2.tile([P, NQ], BF16)
            vs2 = spool2.tile([P, NQ], F32)
            rs2s = spool2.tile([P, NQ], BF16)
            for tb in range(NQ):
                xo = xopool.tile([P, D], F32, tag="xo")
                nc.sync.dma_start(out=xo, in_=x_own[tb * P:(tb + 1) * P, :])
                ps = pr_ps.tile([P, D], F32, tag="pp")
                for pair in range(4):
                    nc.tensor.matmul(
                        ps,
                        o_catT[:, pair, tb * P:(tb + 1) * P],
                        wp_sb[:, pair, :],
                        start=(pair == 0), stop=(pair == 3),
                    )
                nc.vector.tensor_add(x1row[:, tb, :], ps, xo)
                nc.vector.tensor_add(x1row[:, tb, :], x1row[:, tb, :], bpro_bc)
                ln_stats(x1row[:, tb, :], mu2s, vs2, tb, stat2)
            ln_finish(vs2, rs2s)
            nc.sync.dma_start(
                out=mu2v[:].rearrange("(c p) -> p c", p=P), in_=mu2s
            )
            nc.sync.dma_start(
                out=rs2v[:].rearrange("(c p) -> p c", p=P), in_=rs2s
            )

            # transposed projection: x1T = x_ownT + Wproj^T @ o_catT + bproj
            for dt in range(4):
                for tch in range(2):
                    sl = slice(tch * 512, (tch + 1) * 512)
                    psT = prT_ps.tile([P, 512], F32, tag="pt")
                    for pair in range(4):
                        nc.tensor.matmul(
                            psT,
                            wp_sb[:, pair, dt * P:(dt + 1) * P],
                            o_catT[:, pair, sl],
                            start=(pair == 0), stop=(pair == 3),
                        )
                    nc.vector.tensor_scalar(
                        out=x1T[:, dt, sl], in0=psT,
                        scalar1=bpro_sb[:, dt:dt + 1], scalar2=None,
                        op0=mybir.AluOpType.add,
                    )
                    nc.vector.tensor_add(
                        x1T[:, dt, sl], x1T[:, dt, sl], x_ownT_sb[:, dt, sl]
                    )

            mu2_b = bpool2.tile([P, TQ], BF16)
            nc.gpsimd.dma_start(out=mu2_b, in_=mu2v[:].partition_broadcast(P))
            rs2_b = bpool2.tile([P, TQ], BF16)
            nc.gpsimd.dma_start(out=rs2_b, in_=rs2v[:].partition_broadcast(P))
            for c in range(4):
                ln_apply_T(h2T, x1T[:, c, :], mu2_b, rs2_b, c)

        # ---- phase 6: FFN + residual + store -----------------------------
        with ExitStack() as p6:
            f1_ps = p6.enter_context(
                tc.tile_pool(name="f1_ps", bufs=3, space="PSUM")
            )
            f2_ps = p6.enter_context(
                tc.tile_pool(name="f2_ps", bufs=2, space="PSUM")
            )
            fpool = p6.enter_context(tc.tile_pool(name="fpool", bufs=18))
            opool = p6.enter_context(tc.tile_pool(name="opool", bufs=6))
            for tch in range(2):
                tsl = slice(tch * 512, (tch + 1) * 512)
                ff1 = []
                for ht in range(16):
                    ps = f1_ps.tile([P, 512], F32, tag="f1")
                    for c in range(4):
                        nc.tensor.matmul(
                            ps,
                            w1_sb[:, c, ht * P:(ht + 1) * P],
                            h2T[:, c, tsl],
                            start=(c == 0), stop=(c == 3),
                        )
                    f1s = fpool.tile([P, 512], BF16, tag="f1s")
                    nc.vector.tensor_scalar(
                        out=f1s, in0=ps,
                        scalar1=b1_sb[:, ht:ht + 1], scalar2=0.0,
                        op0=mybir.AluOpType.add, op1=mybir.AluOpType.max,
                    )
                    ff1.append(f1s)
                for tbl in range(4):
                    tb = tch * 4 + tbl
                    ps2 = f2_ps.tile([P, D], F32, tag="f2")
                    for ht in range(16):
                        nc.tensor.matmul(
                            ps2,
                            ff1[ht][:, tbl * P:(tbl + 1) * P],
                            w2_sb[:, ht, :],
                            start=(ht == 0), stop=(ht == 15),
                        )
                    orow = opool.tile([P, D], F32, tag="or")
                    nc.vector.tensor_add(orow, ps2, x1row[:, tb, :])
                    nc.vector.tensor_add(orow, orow, b2_bc)
                    nc.sync.dma_start(
                        out=out[tb * P:(tb + 1) * P, :], in_=orow
                    )
    nc.compile()
    return nc


def _make_masks(parity: int) -> np.ndarray:
    """[NT, 128, 256] multiplicative masks restoring exact causality."""
    m = np.zeros((NT, P, 2 * P), np.float32)
    for k in range(NT):
        jmin = JMIN[k]
        for a in range(2):
            g = 2 * (jmin + a) + parity
            t_glob = g * P + np.arange(P)[None, :]
            s_glob = k * P + np.arange(P)[:, None]
            m[k, :, a * P:(a + 1) * P] = (t_glob >= s_glob).astype(np.float32)
    return m.astype(ml_dtypes.bfloat16)


def _prep(inputs):
    f32 = lambda a: np.ascontiguousarray(np.asarray(a, dtype=np.float32))
    bf = lambda a: np.ascontiguousarray(
        np.asarray(a, dtype=np.float32).astype(ml_dtypes.bfloat16)
    )
    x = f32(inputs["x"])
    # [H, D, DH] -> [D, H*DH] with column h*DH+e
    wq = bf(np.asarray(inputs["Wq"], np.float32).transpose(1, 0, 2).reshape(D, D))
    wk = bf(np.asarray(inputs["Wk"], np.float32).transpose(1, 0, 2).reshape(D, D))
    wv = bf(np.asarray(inputs["Wv"], np.float32).transpose(1, 0, 2).reshape(D, D))
    common = {
        "wq": wq, "wk": wk, "wv": wv,
        "wp": bf(inputs["Wproj"]),
        "w1": bf(inputs["W1"]),
        "w2": bf(inputs["W2"]),
        "gvec": f32(inputs["ln1_g"]),
        "bvec": f32(inputs["ln1_b"]),
        "bpro": f32(inputs["bproj"]),
        "b1v": f32(inputs["b1"]),
        "b2v": f32(inputs["b2"]),
    }
    masks = [_make_masks(0), _make_masks(1)]
    in_maps = []
    for c in range(8):
        b, p = c // 2, c % 2
        xb = np.ascontiguousarray(x[b])
        xo = np.ascontiguousarray(
            x[b].reshape(NT, P, D)[p::2].reshape(TQ, D)
        )
        in_maps.append(dict(
            common,
            xb=xb,
            xbT=bf(xb.T),
            x_own=xo,
            x_ownT=bf(xo.T),
            masks=masks[p],
        ))
    return in_maps


def _run(inputs, trace=False):
    if "nc" not in _CACHED:
        _CACHED["nc"] = _build_nc()
    nc = _CACHED["nc"]
    in_maps = _prep(inputs)
    res = run_bass_kernel_spmd(nc, in_maps, core_ids=list(range(8)), trace=trace)
    out = np.empty((B, T, D), np.float32)
    for c in range(8):
        b, p = c // 2, c % 2
        out[b].reshape(NT, P, D)[p::2] = res.results[c]["out"].reshape(NQ, P, D)
    return out, res


def kernel(**inputs) -> np.ndarray:
    out, _ = _run(inputs, trace=False)
    return out


# revision 15
# speedup vs baseline: 1.1347x; 1.0951x over previous
"""Trainium2 Bass kernel for a dense pre-norm transformer block.

Reference semantics (B=4, T=2048, D=512, H=8, DH=64, fp32):
    h  = LN(x; g, b)
    q,k,v = per-head projections of h
    att = causal softmax(q k^T / sqrt(D))
    x1 = x + (att v) @ Wproj + bproj          (heads concatenated)
    h2 = LN(x1; g, b)                         (same LN params, faithful to source)
    out = x1 + relu(h2 @ W1 + b1) @ W2 + b2

Sharding: 8 cores = 4 batches x 2 parities. Core (b, p) owns the 8
row-blocks {p, p+2, ..., p+14} (128 rows each) of batch b. Causal key
extents are rounded up to 512 so even/odd block sets see identical
work -> one uniform SPMD program, no collectives. Exact causality is
restored with multiplicative 0/1 masks on the exp() values (host
provides per-parity masks).

The dataflow needs activations feature-major (features on partitions)
for every matmul, but avoids all on-chip transposes (the DMA-transpose
path only allows 2 sync waits per instruction, which Tile's scheduler
exceeds):
  - the host passes x pre-transposed (xbT, x_ownT, bf16);
  - LN statistics are computed row-major (tokens on partitions, cheap
    free-dim reductions), written to a DRAM scratch row, and read back
    with a 0-stride partition-broadcast DMA so they can be applied in
    the transposed domain;
  - h2T is built from a transposed second projection Wproj^T @ o_catT
    plus the transposed residual, instead of transposing x1.
Scores are computed key-major [s, t]; softmax denominators come for
free from an all-ones column appended to v. All matmuls are bf16 with
fp32 PSUM accumulation; residuals, LN stats and softmax normalization
stay fp32.
"""

import os
import sys

sys.path.insert(0, "/opt/trn_rl_repo")

import numpy as np
import ml_dtypes
from contextlib import ExitStack

import concourse.bass as bass
import concourse.bacc as bacc
import concourse.mybir as mybir
import concourse.tile as tile
from concourse.bass_utils import run_bass_kernel_spmd

B, T, D, H = 4, 2048, 512, 8
DH = D // H            # 64
HID = 4 * D            # 2048
P = 128                # partitions
NT = T // P            # 16 row blocks over full T
NQ = 8                 # own row blocks per core
TQ = NQ * P            # 1024 own rows per core
EPS = 1e-5
SCALE = D ** -0.5
F32 = mybir.dt.float32
BF16 = mybir.dt.bfloat16

# first own-block (local index) attending key-block k; extents rounded to 256
JMIN = [k // 2 for k in range(16)]

_CACHED = {}


def _build_nc():
    nc = bacc.Bacc()

    xbr = nc.dram_tensor("xbr", [T, D], BF16, kind="ExternalInput")
    xbT = nc.dram_tensor("xbT", [D, T], BF16, kind="ExternalInput")
    x_own = nc.dram_tensor("x_own", [TQ, D], F32, kind="ExternalInput")
    x_ownT = nc.dram_tensor("x_ownT", [D, TQ], BF16, kind="ExternalInput")
    wq = nc.dram_tensor("wq", [D, D], BF16, kind="ExternalInput")
    wk = nc.dram_tensor("wk", [D, D], BF16, kind="ExternalInput")
    wv = nc.dram_tensor("wv", [D, D], BF16, kind="ExternalInput")
    wp = nc.dram_tensor("wp", [D, D], BF16, kind="ExternalInput")
    w1 = nc.dram_tensor("w1", [D, HID], BF16, kind="ExternalInput")
    w2 = nc.dram_tensor("w2", [HID, D], BF16, kind="ExternalInput")
    gvec = nc.dram_tensor("gvec", [D], F32, kind="ExternalInput")
    bvec = nc.dram_tensor("bvec", [D], F32, kind="ExternalInput")
    bpro = nc.dram_tensor("bpro", [D], F32, kind="ExternalInput")
    b1v = nc.dram_tensor("b1v", [HID], F32, kind="ExternalInput")
    b2v = nc.dram_tensor("b2v", [D], F32, kind="ExternalInput")
    masks = nc.dram_tensor("masks", [NT, P, P], BF16, kind="ExternalInput")
    out = nc.dram_tensor("out", [TQ, D], F32, kind="ExternalOutput")

    # DRAM scratch: softmax denominators + LN stat rows (for the
    # partition-broadcast round-trips)
    denbuf = nc.dram_tensor("denbuf", [H, TQ], F32)
    muv = nc.dram_tensor("muv", [T], BF16)
    rsv = nc.dram_tensor("rsv", [T], BF16)
    muov = nc.dram_tensor("muov", [TQ], BF16)
    rsov = nc.dram_tensor("rsov", [TQ], BF16)
    mu2v = nc.dram_tensor("mu2v", [TQ], BF16)
    rs2v = nc.dram_tensor("rs2v", [TQ], BF16)

    with ExitStack() as ctx:
        tc = ctx.enter_context(tile.TileContext(nc))
        consts = ctx.enter_context(tc.tile_pool(name="consts", bufs=1))

        # ---- constants ----------------------------------------------------
        g_sb = consts.tile([P, 4], F32)
        nc.sync.dma_start(out=g_sb, in_=gvec[:].rearrange("(c p) -> p c", p=P))
        b_sb = consts.tile([P, 4], F32)
        nc.sync.dma_start(out=b_sb, in_=bvec[:].rearrange("(c p) -> p c", p=P))
        eps_sb = consts.tile([P, 1], F32)
        nc.vector.memset(eps_sb, EPS)

        # ---- persistent activations --------------------------------------
        acts = ctx.enter_context(tc.tile_pool(name="acts", bufs=1))
        x1row = acts.tile([P, NQ, D], F32)
        o_catT = acts.tile([P, 4, TQ], BF16)     # (att@v)^T per head-pair
        h2T = acts.tile([P, 4, TQ], BF16)
        x_ownT_sb = acts.tile([P, 4, TQ], BF16)
        nc.sync.dma_start(
            out=x_ownT_sb, in_=x_ownT[:].rearrange("(c p) t -> p c t", p=P)
        )

        # alive through attention (phases 1-3), freed before FFN
        qkv_pool = ctx.enter_context(tc.tile_pool(name="qkv_pool", bufs=1))
        qT = qkv_pool.tile([P, 4, TQ], BF16)     # own columns only, compact
        kT = qkv_pool.tile([P, 4, T], BF16)
        v_aug = qkv_pool.tile([P, NT, H, DH + 1], BF16)   # v + ones column

        def ln_stats(x_tile, mus, vs_, it, stat_pool):
            """Row-major LN stats of x_tile [128, D] -> mu (bf16) + var col."""
            stats = stat_pool.tile([P, nc.vector.BN_STATS_DIM], F32, tag="st")
            nc.vector.bn_stats(out=stats, in_=x_tile)
            mv = stat_pool.tile([P, nc.vector.BN_AGGR_DIM], F32, tag="mv")
            nc.vector.bn_aggr(out=mv, in_=stats)
            nc.vector.tensor_copy(mus[:, it:it + 1], mv[:, 0:1])
            nc.vector.tensor_copy(vs_[:, it:it + 1], mv[:, 1:2])

        def ln_finish(vs_, rss):
            """rss (bf16) = 1/sqrt(vs_ + eps), one batched op chain."""
            nc.scalar.activation(
                out=vs_, in_=vs_,
                func=mybir.ActivationFunctionType.Sqrt,
                bias=eps_sb, scale=1.0,
            )
            nc.vector.reciprocal(out=vs_, in_=vs_)
            nc.vector.tensor_copy(rss, vs_)

        def ln_apply_T(dst, src_c, mu_b, rs_b, c):
            """dst[:,c,:] = ((src - mu)*rstd)*g + b, transposed domain."""
            nc.vector.tensor_sub(dst[:, c, :], src_c, mu_b)
            nc.vector.tensor_mul(dst[:, c, :], dst[:, c, :], rs_b)
            nc.vector.tensor_scalar(
                out=dst[:, c, :], in0=dst[:, c, :],
                scalar1=g_sb[:, c:c + 1], scalar2=b_sb[:, c:c + 1],
                op0=mybir.AluOpType.mult, op1=mybir.AluOpType.add,
            )

        # ---- phases 1+2: LN1 -> hT -> q/k/v ------------------------------
        with ExitStack() as p12:
            hT_pool = p12.enter_context(tc.tile_pool(name="hT_pool", bufs=1))
            hT = hT_pool.tile([P, 4, T], BF16)       # LN(x)^T, full batch
            hT_own = hT_pool.tile([P, 4, TQ], BF16)  # LN(x)^T, own rows
            xbT_sb = hT_pool.tile([P, 4, T], BF16)
            nc.sync.dma_start(
                out=xbT_sb, in_=xbT[:].rearrange("(c p) t -> p c t", p=P)
            )
            stat1 = p12.enter_context(tc.tile_pool(name="stat1", bufs=8))
            xpool = p12.enter_context(tc.tile_pool(name="xpool", bufs=6))
            spool = p12.enter_context(tc.tile_pool(name="spool", bufs=1))
            bpool = p12.enter_context(tc.tile_pool(name="bpool", bufs=1))

            muso = spool.tile([P, NQ], BF16)
            vso = spool.tile([P, NQ], F32)
            rsso = spool.tile([P, NQ], BF16)
            for it in range(NQ):
                x_tile = xpool.tile([P, D], F32, tag="x")
                nc.sync.dma_start(
                    out=x_tile, in_=x_own[it * P:(it + 1) * P, :]
                )
                ln_stats(x_tile, muso, vso, it, stat1)
            ln_finish(vso, rsso)
            nc.sync.dma_start(
                out=muov[:].rearrange("(c p) -> p c", p=P), in_=muso
            )
            nc.sync.dma_start(
                out=rsov[:].rearrange("(c p) -> p c", p=P), in_=rsso
            )
            muo_b = bpool.tile([P, TQ], BF16)
            nc.gpsimd.dma_start(out=muo_b, in_=muov[:].partition_broadcast(P))
            rso_b = bpool.tile([P, TQ], BF16)
            nc.gpsimd.dma_start(out=rso_b, in_=rsov[:].partition_broadcast(P))
            for c in range(4):
                ln_apply_T(hT_own, x_ownT_sb[:, c, :], muo_b, rso_b, c)

            mus = spool.tile([P, NT], BF16)
            vs1 = spool.tile([P, NT], F32)
            rss = spool.tile([P, NT], BF16)
            for it in range(NT):
                xr_tile = xpool.tile([P, D], BF16, tag="xr")
                nc.sync.dma_start(
                    out=xr_tile, in_=xbr[it * P:(it + 1) * P, :]
                )
                ln_stats(xr_tile, mus, vs1, it, stat1)
            ln_finish(vs1, rss)
            nc.sync.dma_start(out=muv[:].rearrange("(c p) -> p c", p=P), in_=mus)
            nc.sync.dma_start(out=rsv[:].rearrange("(c p) -> p c", p=P), in_=rss)
            mu_b = bpool.tile([P, T], BF16)
            nc.gpsimd.dma_start(out=mu_b, in_=muv[:].partition_broadcast(P))
            rs_b = bpool.tile([P, T], BF16)
            nc.gpsimd.dma_start(out=rs_b, in_=rsv[:].partition_broadcast(P))
            for c in range(4):
                ln_apply_T(hT, xbT_sb[:, c, :], mu_b, rs_b, c)

            # ---- qT / kT / v ---------------------------------------------
            wq_sb = consts.tile([P, 4, D], BF16)
            nc.sync.dma_start(
                out=wq_sb, in_=wq[:].rearrange("(c p) n -> p c n", p=P)
            )
            wk_sb = consts.tile([P, 4, D], BF16)
            nc.sync.dma_start(
                out=wk_sb, in_=wk[:].rearrange("(c p) n -> p c n", p=P)
            )
            wv_sb = consts.tile([P, 4, D], BF16)
            nc.sync.dma_start(
                out=wv_sb, in_=wv[:].rearrange("(c p) n -> p c n", p=P)
            )
            qkv_ps = p12.enter_context(
                tc.tile_pool(name="qkv_ps", bufs=4, space="PSUM")
            )
            for pair in range(4):
                for ts_ in range(2):
                    sl = slice(ts_ * 512, (ts_ + 1) * 512)
                    ps_q = qkv_ps.tile([P, 512], F32, tag="ps")
                    for c in range(4):
                        nc.tensor.matmul(
                            ps_q,
                            wq_sb[:, c, pair * P:(pair + 1) * P],
                            hT_own[:, c, sl],
                            start=(c == 0), stop=(c == 3),
                        )
                    nc.any.tensor_copy(qT[:, pair, sl], ps_q)
            for ts_ in range(4):
                sl = slice(ts_ * 512, (ts_ + 1) * 512)
                for pair in range(4):
                    ps_k = qkv_ps.tile([P, 512], F32, tag="ps")
                    for c in range(4):
                        nc.tensor.matmul(
                            ps_k,
                            wk_sb[:, c, pair * P:(pair + 1) * P],
                            hT[:, c, sl],
                            start=(c == 0), stop=(c == 3),
                        )
                    nc.any.tensor_copy(kT[:, pair, sl], ps_k)
                for st in range(4 * ts_, 4 * ts_ + 4):
                    ps_v = qkv_ps.tile([P, 512], F32, tag="ps")
                    for c in range(4):
                        nc.tensor.matmul(
                            ps_v,
                            hT[:, c, st * P:(st + 1) * P],
                            wv_sb[:, c, :],
                            start=(c == 0), stop=(c == 3),
                        )
                    nc.any.tensor_copy(
                        v_aug[:, st, :, 0:DH],
                        ps_v.rearrange("p (h e) -> p h e", h=H),
                    )
                    nc.vector.memset(v_aug[:, st, :, DH:DH + 1], 1.0)

        # ---- phase 3: attention (head pairs; scores run row-tiled
        # concurrently on the PE for the two heads of a pair) --------------
        masks_sb = consts.tile([P, NT, P], BF16)
        nc.sync.dma_start(out=masks_sb, in_=masks[:].transpose([1, 0, 2]))
        with ExitStack() as p3:
            sc_ps = p3.enter_context(
                tc.tile_pool(name="sc_ps", bufs=2, space="PSUM")
            )
            av_ps = p3.enter_context(
                tc.tile_pool(name="av_ps", bufs=4, space="PSUM")
            )
            epool = p3.enter_context(tc.tile_pool(name="epool", bufs=8))
            dpool = p3.enter_context(tc.tile_pool(name="dpool", bufs=4))
            for pair in range(4):
                prs = [slice(0, DH), slice(DH, 2 * DH)]
                oc00 = av_ps.tile([P, 512], F32, tag="oc")
                oc01 = av_ps.tile([P, 512], F32, tag="oc")
                oc10 = av_ps.tile([P, 512], F32, tag="oc")
                oc11 = av_ps.tile([P, 512], F32, tag="oc")
                ocs = [[oc00, oc01], [oc10, oc11]]   # [half][chunk]
                for k in range(NT):
                    ss = P * JMIN[k]
                    L = TQ - ss
                    sco0 = sc_ps.tile([P, 1024], F32, tag="sc")
                    sco1 = sc_ps.tile([P, 1024], F32, tag="sc")
                    scos = [sco0, sco1]
                    for half in range(2):
                        for n0 in range(0, L, 512):
                            nn = min(512, L - n0)
                            nc.tensor.matmul(
                                scos[half][:, n0:n0 + nn],
                                kT[prs[half], pair, k * P:(k + 1) * P],
                                qT[prs[half], pair, ss + n0:ss + n0 + nn],
                                start=True, stop=True,
                            )
                    for half in range(2):
                        h = 2 * pair + half
                        oc0, oc1 = ocs[half]
                        ex = epool.tile([P, 1024], BF16, tag="ex")
                        nc.scalar.activation(
                            out=ex[:, 0:L], in_=scos[half][:, 0:L],
                            func=mybir.ActivationFunctionType.Exp,
                            scale=SCALE,
                        )
                        nc.vector.tensor_mul(
                            ex[:, 0:P], ex[:, 0:P], masks_sb[:, k, :]
                        )
                        lhs_v = v_aug[:, k, h, :]
                        if ss < 512:
                            nc.tensor.matmul(
                                oc0[0:DH + 1, ss:512],
                                lhs_v,
                                ex[:, 0:512 - ss],
                                start=(k == 0), stop=(k == 7),
                            )
                            nc.tensor.matmul(
                                oc1[0:DH + 1, :],
                                lhs_v,
                                ex[:, 512 - ss:L],
                                start=(k == 0), stop=(k == 15),
                            )
                        else:
                            nc.tensor.matmul(
                                oc1[0:DH + 1, ss - 512:512],
                                lhs_v,
                                ex[:, 0:L],
                                start=False, stop=(k == 15),
                            )
                # softmax denominators -> 1/den, broadcast over 64 partitions
                # (via a DRAM round-trip; SBUF sources reject 0-stride APs)
                for half in range(2):
                    h = 2 * pair + half
                    pr = prs[half]
                    oc0, oc1 = ocs[half]
                    den = dpool.tile([1, TQ], F32, tag="den")
                    nc.vector.reciprocal(den[:, 0:512], oc0[DH:DH + 1, :])
                    nc.vector.reciprocal(den[:, 512:TQ], oc1[DH:DH + 1, :])
                    nc.gpsimd.dma_start(out=denbuf[h, :], in_=den)
                    invb = dpool.tile([DH, TQ], F32, tag="invb")
                    nc.gpsimd.dma_start(
                        out=invb, in_=denbuf[h, :].partition_broadcast(DH)
                    )
                    nc.vector.tensor_mul(
                        o_catT[pr, pair, 0:512], oc0[0:DH, :], invb[:, 0:512]
                    )
                    nc.vector.tensor_mul(
                        o_catT[pr, pair, 512:TQ], oc1[0:DH, :],
                        invb[:, 512:TQ]
                    )

        # ---- phases 4+5: proj (both orientations), residual, LN2 ---------
        wp_sb = consts.tile([P, 4, D], BF16)
        nc.sync.dma_start(
            out=wp_sb, in_=wp[:].rearrange("(c p) n -> p c n", p=P)
        )
        bpro_sb = consts.tile([P, 4], F32)
        nc.sync.dma_start(
            out=bpro_sb, in_=bpro[:].rearrange("(c p) -> p c", p=P)
        )
        bpro_bc = consts.tile([P, D], F32)
        nc.gpsimd.dma_start(out=bpro_bc, in_=bpro[:].partition_broadcast(P))
        with ExitStack() as p45:
            x1T_pool = p45.enter_context(tc.tile_pool(name="x1T_pool", bufs=1))
            x1T = x1T_pool.tile([P, 4, TQ], BF16)
            pr_ps = p45.enter_context(
                tc.tile_pool(name="pr_ps", bufs=2, space="PSUM")
            )
            prT_ps = p45.enter_context(
                tc.tile_pool(name="prT_ps", bufs=2, space="PSUM")
            )
            xopool = p45.enter_context(tc.tile_pool(name="xopool", bufs=6))
            stat2 = p45.enter_context(tc.tile_pool(name="stat2", bufs=8))
            spool2 = p45.enter_context(tc.tile_pool(name="spool2", bufs=1))
            bpool2 = p45.enter_context(tc.tile_pool(name="bpool2", bufs=1))

            mu2s = spool2.tile([P, NQ], BF16)
            vs2 = spool2.tile([P, NQ], F32)
            rs2s = spool2.tile([P, NQ], BF16)
            for tb in range(NQ):
                xo = xopool.tile([P, D], F32, tag="xo")
                nc.sync.dma_start(out=xo, in_=x_own[tb * P:(tb + 1) * P, :])
                ps = pr_ps.tile([P, D], F32, tag="pp")
                for pair in range(4):
                    nc.tensor.matmul(
                        ps,
                        o_catT[:, pair, tb * P:(tb + 1) * P],
                        wp_sb[:, pair, :],
                        start=(pair == 0), stop=(pair == 3),
                    )
                nc.vector.tensor_add(x1row[:, tb, :], ps, xo)
                nc.vector.tensor_add(x1row[:, tb, :], x1row[:, tb, :], bpro_bc)
                ln_stats(x1row[:, tb, :], mu2s, vs2, tb, stat2)
            ln_finish(vs2, rs2s)
            nc.sync.dma_start(
                out=mu2v[:].rearrange("(c p) -> p c", p=P), in_=mu2s
            )
            nc.sync.dma_start(
                out=rs2v[:].rearrange("(c p) -> p c", p=P), in_=rs2s
            )

            # transposed projection: x1T = x_ownT + Wproj^T @ o_catT + bproj
            for dt in range(4):
                for tch in range(2):
                    sl = slice(tch * 512, (tch + 1) * 512)
                    psT = prT_ps.tile([P, 512], F32, tag="pt")
                    for pair in range(4):
                        nc.tensor.matmul(
                            psT,
                            wp_sb[:, pair, dt * P:(dt + 1) * P],
                            o_catT[:, pair, sl],
                            start=(pair == 0), stop=(pair == 3),
                        )
                    nc.vector.tensor_scalar(
                        out=x1T[:, dt, sl], in0=psT,
                        scalar1=bpro_sb[:, dt:dt + 1], scalar2=None,
                        op0=mybir.AluOpType.add,
                    )
                    nc.vector.tensor_add(
                        x1T[:, dt, sl], x1T[:, dt, sl], x_ownT_sb[:, dt, sl]
                    )

            mu2_b = bpool2.tile([P, TQ], BF16)
            nc.gpsimd.dma_start(out=mu2_b, in_=mu2v[:].partition_broadcast(P))
            rs2_b = bpool2.tile([P, TQ], BF16)
            nc.gpsimd.dma_start(out=rs2_b, in_=rs2v[:].partition_broadcast(P))
            for c in range(4):
                ln_apply_T(h2T, x1T[:, c, :], mu2_b, rs2_b, c)

        # ---- phase 6: FFN + residual + store -----------------------------
        w1_sb = consts.tile([P, 4, HID], BF16)
        nc.sync.dma_start(
            out=w1_sb, in_=w1[:].rearrange("(c p) n -> p c n", p=P)
        )
        w2_sb = consts.tile([P, 16, D], BF16)
        nc.sync.dma_start(
            out=w2_sb, in_=w2[:].rearrange("(c p) n -> p c n", p=P)
        )
        b1_sb = consts.tile([P, 16], F32)
        nc.sync.dma_start(out=b1_sb, in_=b1v[:].rearrange("(c p) -> p c", p=P))
        b2_bc = consts.tile([P, D], F32)
        nc.gpsimd.dma_start(out=b2_bc, in_=b2v[:].partition_broadcast(P))
        with ExitStack() as p6:
            f1_ps = p6.enter_context(
                tc.tile_pool(name="f1_ps", bufs=3, space="PSUM")
            )
            f2_ps = p6.enter_context(
                tc.tile_pool(name="f2_ps", bufs=2, space="PSUM")
            )
            fpool = p6.enter_context(tc.tile_pool(name="fpool", bufs=18))
            opool = p6.enter_context(tc.tile_pool(name="opool", bufs=6))
            for tch in range(2):
                tsl = slice(tch * 512, (tch + 1) * 512)
                ff1 = []
                for ht in range(16):
                    ps = f1_ps.tile([P, 512], F32, tag="f1")
                    for c in range(4):
                        nc.tensor.matmul(
                            ps,
                            w1_sb[:, c, ht * P:(ht + 1) * P],
                            h2T[:, c, tsl],
                            start=(c == 0), stop=(c == 3),
                        )
                    f1s = fpool.tile([P, 512], BF16, tag="f1s")
                    # bias+relu+cast on the (otherwise idle) scalar engine
                    nc.scalar.activation(
                        out=f1s, in_=ps,
                        func=mybir.ActivationFunctionType.Relu,
                        bias=b1_sb[:, ht:ht + 1], scale=1.0,
                    )
                    ff1.append(f1s)
                for tbl in range(4):
                    tb = tch * 4 + tbl
                    ps2 = f2_ps.tile([P, D], F32, tag="f2")
                    for ht in range(16):
                        nc.tensor.matmul(
                            ps2,
                            ff1[ht][:, tbl * P:(tbl + 1) * P],
                            w2_sb[:, ht, :],
                            start=(ht == 0), stop=(ht == 15),
                        )
                    orow = opool.tile([P, D], F32, tag="or")
                    nc.vector.tensor_add(orow, ps2, x1row[:, tb, :])
                    nc.vector.tensor_add(orow, orow, b2_bc)
                    nc.sync.dma_start(
                        out=out[tb * P:(tb + 1) * P, :], in_=orow
                    )
    nc.compile()
    return nc


def _make_masks(parity: int) -> np.ndarray:
    """[NT, 128, 128] multiplicative masks for the first suffix block."""
    m = np.zeros((NT, P, P), np.float32)
    for k in range(NT):
        g = 2 * JMIN[k] + parity
        t_glob = g * P + np.arange(P)[None, :]
        s_glob = k * P + np.arange(P)[:, None]
        m[k] = (t_glob >= s_glob).astype(np.float32)
    return m.astype(ml_dtypes.bfloat16)


def _prep(inputs):
    f32 = lambda a: np.ascontiguousarray(np.asarray(a, dtype=np.float32))
    bf = lambda a: np.ascontiguousarray(
        np.asarray(a, dtype=np.float32).astype(ml_dtypes.bfloat16)
    )
    x = f32(inputs["x"])
    # [H, D, DH] -> [D, H*DH] with column h*DH+e
    wq = bf(np.asarray(inputs["Wq"], np.float32).transpose(1, 0, 2).reshape(D, D))
    wk = bf(np.asarray(inputs["Wk"], np.float32).transpose(1, 0, 2).reshape(D, D))
    wv = bf(np.asarray(inputs["Wv"], np.float32).transpose(1, 0, 2).reshape(D, D))
    common = {
        "wq": wq, "wk": wk, "wv": wv,
        "wp": bf(inputs["Wproj"]),
        "w1": bf(inputs["W1"]),
        "w2": bf(inputs["W2"]),
        "gvec": f32(inputs["ln1_g"]),
        "bvec": f32(inputs["ln1_b"]),
        "bpro": f32(inputs["bproj"]),
        "b1v": f32(inputs["b1"]),
        "b2v": f32(inputs["b2"]),
    }
    masks = [_make_masks(0), _make_masks(1)]
    in_maps = []
    for c in range(8):
        b, p = c // 2, c % 2
        xb = np.ascontiguousarray(x[b])
        xo = np.ascontiguousarray(
            x[b].reshape(NT, P, D)[p::2].reshape(TQ, D)
        )
        in_maps.append(dict(
            common,
            xbr=bf(xb),
            xbT=bf(xb.T),
            x_own=xo,
            x_ownT=bf(xo.T),
            masks=masks[p],
        ))
    return in_maps


def _run(inputs, trace=False):
    if "nc" not in _CACHED:
        _CACHED["nc"] = _build_nc()
    nc = _CACHED["nc"]
    in_maps = _prep(inputs)
    res = run_bass_kernel_spmd(nc, in_maps, core_ids=list(range(8)), trace=trace)
    out = np.empty((B, T, D), np.float32)
    for c in range(8):
        b, p = c // 2, c % 2
        out[b].reshape(NT, P, D)[p::2] = res.results[c]["out"].reshape(NQ, P, D)
    return out, res


def kernel(**inputs) -> np.ndarray:
    out, _ = _run(inputs, trace=False)
    return out
